# revision 11
# baseline (speedup 1.0000x reference)
"""Trainium2 Bass kernel v2: sigmoid multi-head attention (16 heads, S=2048,
D=1024, P=64) + final linear, head-sharded across 8 NeuronCores (2 heads/core).

v2 changes vs baseline (all-fp32r):
  * fp16 operands for Q/K projections, scores, and final linear (1 c/row on PE,
    half the DMA + SBUF traffic of fp32).
  * V path: natural-layout V computed directly (no PE transposes) with fp8e4
    DoubleRow matmuls (K=256/pass). The per-head V block is DUPLICATED on both
    64-col halves of the stationary tile, so the attn output lands identically
    on both PSUM partition halves - giving a free partition-replicated attnT
    for the final-stage staging (no cross-partition copies).
  * attn matmuls in fp8e4 DoubleRow (sigmoid emits fp8 directly): two t-tiles
    (K=256) per pass.
  * final linear restructured to K=128 (u-parity x p stacked on partitions) and
    M=128 (both heads' 64 output rows), 4x fewer PE rows than baseline.

Per-core output rows [256c, 256c+256) exactly (head h = row // 128), host
gather is a concatenation.

Numerics (numpy-simulated): rel err ~4e-3 vs fp32 reference (gate 2e-2).
"""

import os

os.environ.setdefault("BASS_NEVER_TRACE", "1")

import numpy as np
from contextlib import ExitStack

import jax
import concourse.bacc as bacc
import concourse.bass as bass
import concourse.mybir as mybir
import concourse.tile as tile
from concourse.bass2jax import (
    _bass_exec_p,
    install_neuronx_cc_hook,
    partition_id_tensor,
)
from jax.experimental.shard_map import shard_map
from jax.sharding import Mesh, NamedSharding, PartitionSpec

S, D, H, P, F = 2048, 1024, 16, 64, 1024
NCORES = 8
HL = H // NCORES          # heads per core = 2
P2 = HL * P               # stacked head dim = 128
DCH = D // 128            # 8 contraction chunks
NSB = S // 512            # 4 s-blocks
NT = S // 128             # 16 t-tiles
NI = 8                    # final-linear accumulation steps (u-pairs)

FP32 = mybir.dt.float32
FP16 = mybir.dt.float16
FP8 = mybir.dt.float8e4
SIGMOID = mybir.ActivationFunctionType.Sigmoid
DR = mybir.MatmulPerfMode.DoubleRow

# timing-ablation level (outputs become garbage; timing probes only):
# 5=full, 4=drop final linear+output, 3=+drop attn+staging, 2=+drop sigmoids,
# 1=+drop scores, 0=input DMAs only
KLEVEL = int(os.environ.get("KLEVEL", "5"))


def build_kernel(ctx: ExitStack, tc: tile.TileContext, xt_d, xt8_d, wq_d, wk_d,
                 wv8_d, wf3_d, out_d):
    nc = tc.nc

    w_pool = ctx.enter_context(tc.tile_pool(name="wts", bufs=1))
    qk_pool = ctx.enter_context(tc.tile_pool(name="qk", bufs=1))

    qt2 = qk_pool.tile([128, S], FP16, tag="qt2")        # [p2, s]
    kt2 = qk_pool.tile([128, S], FP16, tag="kt2")        # [p2, t]
    # natural V, j-major, per-head duplicated halves:
    # v2n[t_in, j, q2*128 + dup*64 + p] = V[head q2, j*128+t_in, p]
    v2n = qk_pool.tile([128, NT, 256], FP8, tag="v2n")

    # PSUM plan (8 banks): stage-P uses 4 (pj x2 + pv x2, right side, released
    # before stage S). Stage S: ps_s double-buffered [128,1024] = 4 banks,
    # ps_a (a0+a1) = 2 banks, psf double-buffered = 2 banks from the released
    # stage-P zone.
    pp_pool = tc.alloc_tile_pool(name="pp", bufs=1, space="PSUM", side="right")
    ps_s_pool = ctx.enter_context(tc.tile_pool(name="ps_s", bufs=2, space="PSUM"))
    ps_a_pool = ctx.enter_context(tc.tile_pool(name="ps_a", bufs=1, space="PSUM"))
    sc_pool = ctx.enter_context(tc.tile_pool(name="sc", bufs=3))
    at_pool = ctx.enter_context(tc.tile_pool(name="att", bufs=2))
    ot_pool = ctx.enter_context(tc.tile_pool(name="ot", bufs=4))

    def emit_scores_attn(sb_s, jp, ps_a, sc_pool):
        """Scores + sigmoid + fp8-DR attn accumulation for query s-block
        `sb_s`, t-tile pair `jp`."""
        if KLEVEL < 2:
            return
        s0 = sb_s * 512
        sc = (sc_pool.tile([128, 2, 1024], FP8, tag="sc",
                           name=f"sc{sb_s}_{jp}")
              if KLEVEL >= 3 else None)
        for kt in range(2):
            j = 2 * jp + kt
            t0 = j * 128
            ps_s = ps_s_pool.tile([128, 1024], FP32, tag="ps_s",
                                  name=f"ps_s{sb_s}_{j}")
            # scoreT h0 / h1 - concurrent on PE rows 0:64 / 64:128
            if KLEVEL >= 2:
                nc.tensor.matmul(ps_s[:, 0:512],
                                 kt2[0:64, t0:t0 + 128],
                                 qt2[0:64, s0:s0 + 512])
                nc.tensor.matmul(ps_s[:, 512:1024],
                                 kt2[64:128, t0:t0 + 128],
                                 qt2[64:128, s0:s0 + 512])
            if KLEVEL >= 3:
                nc.scalar.activation(sc[:, kt:kt + 1, :], ps_s,
                                     SIGMOID, scale=1.0 / P)
        # attnT accumulation, fp8 DoubleRow over the j-pair
        for h in range(2):
            if KLEVEL < 4:
                break
            nc.tensor.matmul(ps_a[h],
                             v2n[:, 2 * jp:2 * jp + 2,
                                 h * 128:(h + 1) * 128],
                             sc[:, 0:2, h * 512:(h + 1) * 512],
                             start=(jp == 0),
                             stop=(jp == NT // 2 - 1),
                             perf_mode=DR)

    def alloc_ps_a(sb_s):
        # ps_a[h]: [128, r' 32, u 16] == [p2, s_local 512]
        return [ps_a_pool.tile([128, 32, 16], FP32, tag=f"a{h}",
                               name=f"ps_a{h}_{sb_s}")
                for h in range(2)]

    def emit_at2q_copies(at2q, ps_a, half):
        # stage into at2q: partition half a holds u-parity a (the duplicated
        # attn halves make both a-halves partition-aligned)
        if KLEVEL < 4:
            return
        for h in range(2):
            for a in range(2):
                nc.vector.tensor_copy(
                    at2q[a * 64:(a + 1) * 64, 0:NI,
                         h * 64 + 32 * half:h * 64 + 32 * half + 32],
                    ps_a[h][a * 64:(a + 1) * 64, 0:32,
                            a::2].transpose([0, 2, 1]))

    def emit_final(q, at2q, ps_f_pool):
        # final linear: K=128 ((a,p) on partitions), M=128 (both heads)
        if KLEVEL < 5:
            return
        for fc in range(2):
            psf = ps_f_pool.tile([128, 512], FP32, tag="pf",
                                 name=f"psf{q}_{fc}")
            for i in range(NI):
                nc.tensor.matmul(psf, at2q[:, i:i + 1, 0:128],
                                 wf3[:, i:i + 1, fc * 512:(fc + 1) * 512],
                                 start=(i == 0), stop=(i == NI - 1))
            otf = ot_pool.tile([128, 512], FP32, tag="ot",
                               name=f"ot{q}_{fc}")
            nc.vector.tensor_copy(otf, psf)
            for h in range(2):
                nc.sync.dma_start(
                    out_d[h * 128 + 64 * q: h * 128 + 64 * (q + 1),
                          fc * 512:(fc + 1) * 512],
                    otf[h * 64:(h + 1) * 64, :])
    # (ablation levels below 5 emit no final/otf/out-DMA)

    # -------- stage P: projections, s-column-streamed, interleaved with the
    # q0/half0 score+sigmoid+attn stream (the sigmoid stream is the kernel's
    # throughput bound - start it as early as possible) --------
    at2q0 = at_pool.tile([128, NI, 128], FP16, tag="at", name="at0")
    ps_a_q0 = alloc_ps_a(0)

    with tc.tile_pool(name="xt", bufs=2) as xt_pool, \
         tc.tile_pool(name="xt8", bufs=2) as xt8_pool:

        # small weight DMAs on SWDGE queues; the x stream owns HWDGE from t=0
        wq = w_pool.tile([128, D], FP16, tag="wq")
        nc.gpsimd.dma_start(wq, wq_d)
        wk = w_pool.tile([128, D], FP16, tag="wk")
        nc.gpsimd.dma_start(wk, wk_d)
        wv8 = w_pool.tile([128, DCH, 256], FP8, tag="wv8")
        nc.gpsimd.dma_start(wv8, wv8_d.bitcast(FP8))

        xts = [[None] * DCH for _ in range(NSB)]
        xt8s = [None] * NSB
        for sb in range(NSB):
            for d in range(DCH):
                xt_t = xt_pool.tile([128, 512], FP16, tag=f"x{d}",
                                    name=f"xt{sb}_{d}")
                nc.sync.dma_start(
                    xt_t,
                    xt_d[d * 128:(d + 1) * 128, sb * 512:(sb + 1) * 512])
                xts[sb][d] = xt_t
            x8_t = xt8_pool.tile([128, DCH, 512], FP8, tag="x8",
                                 name=f"xt8_{sb}")
            nc.sync.dma_start(
                x8_t, xt8_d[:, sb * DCH:(sb + 1) * DCH, :].bitcast(FP8))
            xt8s[sb] = x8_t

        # wf3 [128, NI, F] fp16: row a*64+p, chunk i -> W_fin[(2i+a)*64+p, :].
        # Emitted after xt so the x stream wins the DMA queues.
        wf3 = w_pool.tile([128, NI, F], FP16, tag="wf3")
        for i in range(4):
            nc.sync.dma_start(wf3[:, 2 * i:2 * i + 2, :],
                              wf3_d[:, 2 * i:2 * i + 2, :])

        for sb in range(NSB):
            cols = slice(sb * 512, (sb + 1) * 512)
            for w, dst in ((wk, kt2), (wq, qt2)):
                if KLEVEL < 1:
                    break
                ps_p = pp_pool.tile([128, 512], FP32, tag="pj",
                                    name=f"pp{sb}_{0 if dst is kt2 else 1}")
                for d in range(DCH):
                    nc.tensor.matmul(ps_p, w[:, d * 128:(d + 1) * 128],
                                     xts[sb][d],
                                     start=(d == 0), stop=(d == DCH - 1))
                nc.vector.tensor_copy(dst[:, cols], ps_p)
            # natural V via fp8 DoubleRow: per t-tile, K=256 per pass
            for jj in range(4):
                if KLEVEL < 1:
                    break
                j = sb * 4 + jj
                t0 = jj * 128
                pv = pp_pool.tile([128, 256], FP32, tag="pv", name=f"pv{j}")
                for m in range(DCH // 2):
                    nc.tensor.matmul(pv,
                                     xt8s[sb][:, 2 * m:2 * m + 2, t0:t0 + 128],
                                     wv8[:, 2 * m:2 * m + 2, :],
                                     start=(m == 0), stop=(m == DCH // 2 - 1),
                                     perf_mode=DR)
                nc.vector.tensor_copy(v2n[:, j:j + 1, :], pv)
            # q0/half0 scores over this s-block's freshly-projected t-tiles
            for jp in (2 * sb, 2 * sb + 1):
                emit_scores_attn(0, jp, ps_a_q0, sc_pool)

    pp_pool.release()

    # ---------------- stage S + F: remaining scores / attn / final ----------
    with tc.tile_pool(name="ps_f", bufs=2, space="PSUM", side="right") as ps_f_pool:

        emit_at2q_copies(at2q0, ps_a_q0, 0)
        # q0 half1
        ps_a = alloc_ps_a(1)
        for jp in range(NT // 2):
            emit_scores_attn(1, jp, ps_a, sc_pool)
        emit_at2q_copies(at2q0, ps_a, 1)
        emit_final(0, at2q0, ps_f_pool)

        # q1
        at2q1 = at_pool.tile([128, NI, 128], FP16, tag="at", name="at1")
        for half in range(2):
            sb_s = 2 + half
            ps_a = alloc_ps_a(sb_s)
            for jp in range(NT // 2):
                emit_scores_attn(sb_s, jp, ps_a, sc_pool)
            emit_at2q_copies(at2q1, ps_a, half)
        emit_final(1, at2q1, ps_f_pool)


def build_bass(replicas: int = 1) -> bass.Bass:
    nc = bacc.Bacc("TRN2", target_bir_lowering=False, debug=False,
                   num_devices=NCORES)
    xt_d = nc.dram_tensor("xt", [D, S], FP16, kind="ExternalInput").ap()
    xt8_d = nc.dram_tensor("xt8", [128, NSB * DCH, 512], mybir.dt.uint8,
                           kind="ExternalInput").ap()
    wq_d = nc.dram_tensor("wq", [128, D], FP16, kind="ExternalInput").ap()
    wk_d = nc.dram_tensor("wk", [128, D], FP16, kind="ExternalInput").ap()
    wv8_d = nc.dram_tensor("wv8", [128, DCH, 256], mybir.dt.uint8,
                           kind="ExternalInput").ap()
    wf3_d = nc.dram_tensor("wf3", [128, NI, F], FP16,
                           kind="ExternalInput").ap()
    out_d = nc.dram_tensor("out", [HL * 128, F], FP32,
                           kind="ExternalOutput").ap()
    with tile.TileContext(nc) as tc:
        for _ in range(replicas):
            with ExitStack() as ctx:
                build_kernel(ctx, tc, xt_d, xt8_d, wq_d, wk_d, wv8_d, wf3_d,
                             out_d)
    nc.finalize()
    return nc


_NC_CACHE = None
_EXEC_CACHE = None
LAST_DEV_ARGS = None
LAST_OUT_NAMES = None


def _get_nc():
    global _NC_CACHE
    if _NC_CACHE is None:
        _NC_CACHE = build_bass()
    return _NC_CACHE


def _get_executor():
    """Compile the SPMD PJRT executable once (mirrors bass2jax.run_bass_via_pjrt,
    minus output-buffer donation)."""
    global _EXEC_CACHE
    if _EXEC_CACHE is not None:
        return _EXEC_CACHE
    import concourse.mybir as mybir

    nc = _get_nc()
    install_neuronx_cc_hook()
    partition_name = (nc.partition_id_tensor.name
                      if nc.partition_id_tensor else None)
    in_names, out_names, out_avals = [], [], []
    out_shapes = []
    for alloc in nc.m.functions[0].allocations:
        if not isinstance(alloc, mybir.MemoryLocationSet):
            continue
        name = alloc.memorylocations[0].name
        if alloc.kind == "ExternalInput":
            if name != partition_name:
                in_names.append(name)
        elif alloc.kind == "ExternalOutput":
            shape = tuple(alloc.tensor_shape)
            dtype = mybir.dt.np(alloc.dtype)
            out_names.append(name)
            out_avals.append(jax.core.ShapedArray(shape, dtype))
            out_shapes.append((shape, dtype))
    n_params = len(in_names)
    all_names = list(in_names) + list(out_names)
    if partition_name is not None:
        all_names.append(partition_name)

    def _body(*args):
        operands = list(args)
        if partition_name is not None:
            operands.append(partition_id_tensor())
        outs = _bass_exec_p.bind(
            *operands,
            out_avals=tuple(out_avals),
            in_names=tuple(all_names),
            out_names=tuple(out_names),
            lowering_input_output_aliases=(),
            sim_require_finite=True,
            sim_require_nnan=True,
            nc=nc,
        )
        return tuple(outs)

    devices = jax.devices()[:NCORES]
    mesh = Mesh(np.asarray(devices), ("core",))
    n_args = n_params + len(out_names)
    sharded = jax.jit(shard_map(
        _body, mesh=mesh,
        in_specs=(PartitionSpec("core"),) * n_args,
        out_specs=(PartitionSpec("core"),) * len(out_names),
        check_rep=False))
    _EXEC_CACHE = (sharded, mesh, in_names, out_names, out_shapes)
    return _EXEC_CACHE


def _run_spmd(in_maps):
    """Execute on all cores; returns list of per-core {name: np.ndarray}."""
    global LAST_DEV_ARGS, LAST_OUT_NAMES
    sharded, mesh, in_names, out_names, out_shapes = _get_executor()
    sh = NamedSharding(mesh, PartitionSpec("core"))
    args = [np.concatenate([im[name] for im in in_maps], axis=0)
            for name in in_names]
    for shape, dtype in out_shapes:
        args.append(np.zeros((NCORES * shape[0],) + shape[1:], dtype))
    dev_args = [jax.device_put(a, sh) for a in args]
    LAST_DEV_ARGS = dev_args
    LAST_OUT_NAMES = out_names
    outs = sharded(*dev_args)
    jax.block_until_ready(outs)
    results = []
    for c in range(NCORES):
        res = {}
        for i, name in enumerate(out_names):
            g = np.asarray(outs[i])
            d0 = g.shape[0] // NCORES
            res[name] = g[c * d0:(c + 1) * d0]
        results.append(res)
    return results


def bench(iters=32):
    import time
    sharded = _get_executor()[0]
    assert LAST_DEV_ARGS is not None, "call kernel() first"
    outs = sharded(*LAST_DEV_ARGS)
    jax.block_until_ready(outs)
    t0 = time.perf_counter()
    pend = [sharded(*LAST_DEV_ARGS) for _ in range(iters)]
    jax.block_until_ready(pend)
    return (time.perf_counter() - t0) / iters


_FAST_CACHE = None


def _get_fast():
    global _FAST_CACHE
    if _FAST_CACHE is not None:
        return _FAST_CACHE
    _FAST_CACHE = _make_fast_for(_get_nc())
    return _FAST_CACHE


def bench_fast(iters=64):
    import time
    fn = _get_fast()
    assert LAST_DEV_ARGS is not None
    outs = fn(*LAST_DEV_ARGS)
    jax.block_until_ready(outs)
    best = None
    for _ in range(3):
        t0 = time.perf_counter()
        pend = [fn(*LAST_DEV_ARGS) for _ in range(iters)]
        jax.block_until_ready(pend)
        dt = (time.perf_counter() - t0) / iters
        best = dt if best is None else min(best, dt)
    return best


def _make_fast_for(nc):
    from concourse.bass2jax import fast_dispatch_compile
    import concourse.mybir as mybir

    install_neuronx_cc_hook()
    pn = nc.partition_id_tensor.name if nc.partition_id_tensor else None
    in_names, out_names, out_avals = [], [], []
    for alloc in nc.m.functions[0].allocations:
        if not isinstance(alloc, mybir.MemoryLocationSet):
            continue
        name = alloc.memorylocations[0].name
        if alloc.kind == "ExternalInput":
            if name != pn:
                in_names.append(name)
        elif alloc.kind == "ExternalOutput":
            out_names.append(name)
            out_avals.append(jax.core.ShapedArray(
                tuple(alloc.tensor_shape), mybir.dt.np(alloc.dtype)))
    all_names = list(in_names) + list(out_names)
    if pn:
        all_names.append(pn)

    def _body(*a):
        ops = list(a)
        if pn:
            ops.append(partition_id_tensor())
        return tuple(_bass_exec_p.bind(
            *ops, out_avals=tuple(out_avals), in_names=tuple(all_names),
            out_names=tuple(out_names), lowering_input_output_aliases=(),
            sim_require_finite=True, sim_require_nnan=True, nc=nc))

    mesh = Mesh(np.asarray(jax.devices()[:NCORES]), ("core",))
    na = len(in_names) + len(out_names)

    def cf():
        return jax.jit(shard_map(
            _body, mesh=mesh,
            in_specs=(PartitionSpec("core"),) * na,
            out_specs=(PartitionSpec("core"),) * len(out_names),
            check_rep=False)).lower(*LAST_DEV_ARGS).compile()

    return fast_dispatch_compile(cf)


def bench_body(rounds=9, n=96, r_lo=3, r_hi=15):
    """Kernel-body execution time via the replica-count slope: per-exec wall
    time of an r_hi-body NEFF minus an r_lo-body NEFF, divided by the body
    delta. Rounds alternate lo/hi so slow tunnel drift cancels; the median
    difference is returned. Multi-body NEFFs keep the device busy well past
    the ~1 ms axon dispatch overhead."""
    import time
    import statistics
    assert LAST_DEV_ARGS is not None, "call kernel() first"
    fn_lo = _make_fast_for(build_bass(replicas=r_lo))
    fn_hi = _make_fast_for(build_bass(replicas=r_hi))
    jax.block_until_ready(fn_lo(*LAST_DEV_ARGS))
    jax.block_until_ready(fn_hi(*LAST_DEV_ARGS))

    def run(fn, m):
        t0 = time.perf_counter()
        pend = [fn(*LAST_DEV_ARGS) for _ in range(m)]
        jax.block_until_ready(pend)
        return (time.perf_counter() - t0) / m

    run(fn_lo, 16)
    run(fn_hi, 16)
    diffs = []
    for _ in range(rounds):
        lo = run(fn_lo, n)
        hi = run(fn_hi, n)
        diffs.append((hi - lo) / (r_hi - r_lo))
    return statistics.median(diffs)


def _np_fp8():
    return mybir.dt.np(FP8)


def _layout_w(w, c):
    """[H, D, P] global weights -> per-core [128, D] stationary layout:
    out[di, dc*128 + (h*64+p)] = w[2c+h, dc*128+di, p]"""
    wl = np.transpose(w[HL * c:HL * (c + 1)], (1, 0, 2)).reshape(D, P2)
    wl = wl.reshape(DCH, 128, P2).transpose(1, 0, 2).reshape(128, DCH * P2)
    return np.ascontiguousarray(wl, dtype=np.float16)


def make_in_maps(x, Qw, Kw, Vw, W_fin):
    f8 = _np_fp8()
    xt16 = np.ascontiguousarray(x.T.astype(np.float16))          # [D, S]
    # xt8[di, sb*8+ch, s'] = x[sb*512+s', ch*128+di]
    x8 = x.astype(f8)                                            # [S, D]
    xt8 = np.ascontiguousarray(
        x8.reshape(NSB, 512, DCH, 128).transpose(3, 0, 2, 1)
        .reshape(128, NSB * DCH, 512)).view(np.uint8)
    # wf3[a*64+p, i, f] = W_fin[(2i+a)*64+p, f]
    wf = W_fin.reshape(NI, 2, 64, F).transpose(1, 2, 0, 3)       # [a, p, i, f]
    wf3 = np.ascontiguousarray(wf.reshape(128, NI, F).astype(np.float16))

    in_maps = []
    for c in range(NCORES):
        # wv8[di, ch, q2*128 + dup*64 + p] = Vw[2c+q2, ch*128+di, p]
        vws = Vw[HL * c:HL * c + HL].astype(f8)                  # [2, D, P]
        vv = vws.reshape(HL, DCH, 128, P).transpose(2, 1, 0, 3)  # [di, ch, q2, p]
        wv8 = np.empty((128, DCH, HL, 2, P), f8)
        wv8[:, :, :, 0, :] = vv
        wv8[:, :, :, 1, :] = vv
        wv8 = np.ascontiguousarray(wv8.reshape(128, DCH, 256)).view(np.uint8)
        in_maps.append({
            "xt": xt16,
            "xt8": xt8,
            "wq": _layout_w(Qw, c),
            "wk": _layout_w(Kw, c),
            "wv8": wv8,
            "wf3": wf3,
        })
    return in_maps


def kernel(x, Qw, Kw, Vw, W_fin, b_fin):
    x = np.asarray(x, dtype=np.float32)
    Qw = np.asarray(Qw, dtype=np.float32)
    Kw = np.asarray(Kw, dtype=np.float32)
    Vw = np.asarray(Vw, dtype=np.float32)
    W_fin = np.asarray(W_fin, dtype=np.float32)
    b_fin = np.asarray(b_fin, dtype=np.float32)

    in_maps = make_in_maps(x, Qw, Kw, Vw, W_fin)
    results = _run_spmd(in_maps)
    out = np.concatenate([results[c]["out"] for c in range(NCORES)], axis=0)
    return (out + b_fin).astype(np.float32)


# revision 12
# speedup vs baseline: 1.2869x; 1.2869x over previous
"""Trainium2 Bass kernel v2: sigmoid multi-head attention (16 heads, S=2048,
D=1024, P=64) + final linear, head-sharded across 8 NeuronCores (2 heads/core).

v2 changes vs baseline (all-fp32r):
  * fp16 operands for Q/K projections, scores, and final linear (1 c/row on PE,
    half the DMA + SBUF traffic of fp32).
  * V path: natural-layout V computed directly (no PE transposes) with fp8e4
    DoubleRow matmuls (K=256/pass). The per-head V block is DUPLICATED on both
    64-col halves of the stationary tile, so the attn output lands identically
    on both PSUM partition halves - giving a free partition-replicated attnT
    for the final-stage staging (no cross-partition copies).
  * attn matmuls in fp8e4 DoubleRow (sigmoid emits fp8 directly): two t-tiles
    (K=256) per pass.
  * final linear restructured to K=128 (u-parity x p stacked on partitions) and
    M=128 (both heads' 64 output rows), 4x fewer PE rows than baseline.

Per-core output rows [256c, 256c+256) exactly (head h = row // 128), host
gather is a concatenation.

Numerics (numpy-simulated): rel err ~4e-3 vs fp32 reference (gate 2e-2).
"""

import os

os.environ.setdefault("BASS_NEVER_TRACE", "1")

import numpy as np
from contextlib import ExitStack

import jax
import concourse.bacc as bacc
import concourse.bass as bass
import concourse.mybir as mybir
import concourse.tile as tile
from concourse.bass2jax import (
    _bass_exec_p,
    install_neuronx_cc_hook,
    partition_id_tensor,
)
from jax.experimental.shard_map import shard_map
from jax.sharding import Mesh, NamedSharding, PartitionSpec

S, D, H, P, F = 2048, 1024, 16, 64, 1024
NCORES = 8
HL = H // NCORES          # heads per core = 2
P2 = HL * P               # stacked head dim = 128
DCH = D // 128            # 8 contraction chunks
NSB = S // 512            # 4 s-blocks
NT = S // 128             # 16 t-tiles
NI = 8                    # final-linear accumulation steps (u-pairs)

FP32 = mybir.dt.float32
FP16 = mybir.dt.float16
FP8 = mybir.dt.float8e4
SIGMOID = mybir.ActivationFunctionType.Sigmoid
DR = mybir.MatmulPerfMode.DoubleRow

# timing-ablation level (outputs become garbage; timing probes only):
# 5=full, 4=drop final linear+output, 3=+drop attn+staging, 2=+drop sigmoids,
# 1=+drop scores, 0=input DMAs only
KLEVEL = int(os.environ.get("KLEVEL", "5"))


def build_kernel(ctx: ExitStack, tc: tile.TileContext, xt_d, xt8_d, wq_d, wk_d,
                 wv8_d, wf3_d, out_d):
    nc = tc.nc

    w_pool = ctx.enter_context(tc.tile_pool(name="wts", bufs=1))
    qk_pool = ctx.enter_context(tc.tile_pool(name="qk", bufs=1))

    qt2 = qk_pool.tile([128, S], FP16, tag="qt2")        # [p2, s]
    kt2 = qk_pool.tile([128, S], FP16, tag="kt2")        # [p2, t]
    # natural V, j-major, per-head duplicated halves:
    # v2n[t_in, j, q2*128 + dup*64 + p] = V[head q2, j*128+t_in, p]
    v2n = qk_pool.tile([128, NT, 256], FP8, tag="v2n")

    # PSUM plan (8 banks): stage-P uses 4 (pj x2 + pv x2, right side, released
    # before stage S). Stage S: ps_s double-buffered [128,1024] = 4 banks,
    # ps_a (a0+a1) = 2 banks, psf double-buffered = 2 banks from the released
    # stage-P zone.
    pp_pool = tc.alloc_tile_pool(name="pp", bufs=1, space="PSUM", side="right")
    ps_s_pool = ctx.enter_context(tc.tile_pool(name="ps_s", bufs=2, space="PSUM"))
    ps_a_pool = ctx.enter_context(tc.tile_pool(name="ps_a", bufs=1, space="PSUM"))
    sc_pool = ctx.enter_context(tc.tile_pool(name="sc", bufs=3))
    at_pool = ctx.enter_context(tc.tile_pool(name="att", bufs=2))
    ot_pool = ctx.enter_context(tc.tile_pool(name="ot", bufs=4))

    def emit_scores_attn(sb_s, jp, ps_a, sc_pool):
        """Scores + sigmoid + fp8-DR attn accumulation for query s-block
        `sb_s`, t-tile pair `jp`."""
        if KLEVEL < 2:
            return
        s0 = sb_s * 512
        sc = (sc_pool.tile([128, 2, 1024], FP8, tag="sc",
                           name=f"sc{sb_s}_{jp}")
              if KLEVEL >= 3 else None)
        for kt in range(2):
            j = 2 * jp + kt
            t0 = j * 128
            ps_s = ps_s_pool.tile([128, 1024], FP32, tag="ps_s",
                                  name=f"ps_s{sb_s}_{j}")
            # scoreT h0 / h1 - concurrent on PE rows 0:64 / 64:128
            if KLEVEL >= 2:
                nc.tensor.matmul(ps_s[:, 0:512],
                                 kt2[0:64, t0:t0 + 128],
                                 qt2[0:64, s0:s0 + 512])
                nc.tensor.matmul(ps_s[:, 512:1024],
                                 kt2[64:128, t0:t0 + 128],
                                 qt2[64:128, s0:s0 + 512])
            if KLEVEL >= 3:
                nc.scalar.activation(sc[:, kt:kt + 1, :], ps_s,
                                     SIGMOID, scale=1.0 / P)
        # attnT accumulation, fp8 DoubleRow over the j-pair
        for h in range(2):
            if KLEVEL < 4:
                break
            nc.tensor.matmul(ps_a[h],
                             v2n[:, 2 * jp:2 * jp + 2,
                                 h * 128:(h + 1) * 128],
                             sc[:, 0:2, h * 512:(h + 1) * 512],
                             start=(jp == 0),
                             stop=(jp == NT // 2 - 1),
                             perf_mode=DR)

    def alloc_ps_a(sb_s):
        # ps_a[h]: [128, r' 32, u 16] == [p2, s_local 512]
        return [ps_a_pool.tile([128, 32, 16], FP32, tag=f"a{h}",
                               name=f"ps_a{h}_{sb_s}")
                for h in range(2)]

    def emit_at2q_copies(at2q, ps_a, half):
        # stage into at2q: partition half a holds u-parity a (the duplicated
        # attn halves make both a-halves partition-aligned)
        if KLEVEL < 4:
            return
        for h in range(2):
            for a in range(2):
                nc.vector.tensor_copy(
                    at2q[a * 64:(a + 1) * 64, 0:NI,
                         h * 64 + 32 * half:h * 64 + 32 * half + 32],
                    ps_a[h][a * 64:(a + 1) * 64, 0:32,
                            a::2].transpose([0, 2, 1]))

    def emit_final(q, at2q, ps_f_pool):
        # final linear: K=128 ((a,p) on partitions), M=128 (both heads)
        if KLEVEL < 5:
            return
        for fc in range(2):
            psf = ps_f_pool.tile([128, 512], FP32, tag="pf",
                                 name=f"psf{q}_{fc}")
            for i in range(NI):
                nc.tensor.matmul(psf, at2q[:, i:i + 1, 0:128],
                                 wf3[:, i:i + 1, fc * 512:(fc + 1) * 512],
                                 start=(i == 0), stop=(i == NI - 1))
            otf = ot_pool.tile([128, 512], FP16, tag="ot",
                               name=f"ot{q}_{fc}")
            nc.vector.tensor_copy(otf, psf)
            for h in range(2):
                nc.sync.dma_start(
                    out_d[h * 128 + 64 * q: h * 128 + 64 * (q + 1),
                          fc * 512:(fc + 1) * 512],
                    otf[h * 64:(h + 1) * 64, :])
    # (ablation levels below 5 emit no final/otf/out-DMA)

    # -------- stage P: projections, s-column-streamed, interleaved with the
    # q0/half0 score+sigmoid+attn stream (the sigmoid stream is the kernel's
    # throughput bound - start it as early as possible) --------
    at2q0 = at_pool.tile([128, NI, 128], FP16, tag="at", name="at0")
    ps_a_q0 = alloc_ps_a(0)

    # PE p-state warmup: dummy matmuls with no DMA dependency ramp the PE
    # clock (0.65 -> 2.4 GHz takes ~3 us busy) while the x stream arrives
    if KLEVEL >= 1:
        with tc.tile_pool(name="wu", bufs=1) as wu_pool:
            wsrc = wu_pool.tile([128, 512], FP16, tag="ws")
            nc.vector.memset(wsrc, 0)
            wp = ps_s_pool.tile([128, 512], FP32, tag="ps_s", name="warm")
            for i in range(12):
                nc.tensor.matmul(wp, wsrc[:, 0:128], wsrc,
                                 start=True, stop=True)

    with tc.tile_pool(name="xt", bufs=2) as xt_pool, \
         tc.tile_pool(name="xt8", bufs=2) as xt8_pool:

        # small weight DMAs on SWDGE queues; the x stream owns HWDGE from t=0
        wq = w_pool.tile([128, D], FP16, tag="wq")
        nc.gpsimd.dma_start(wq, wq_d)
        wk = w_pool.tile([128, D], FP16, tag="wk")
        nc.gpsimd.dma_start(wk, wk_d)
        wv8 = w_pool.tile([128, DCH, 256], FP8, tag="wv8")
        nc.gpsimd.dma_start(wv8, wv8_d.bitcast(FP8))

        xts = [[None] * DCH for _ in range(NSB)]
        xt8s = [None] * NSB
        for sb in range(NSB):
            for d in range(DCH):
                xt_t = xt_pool.tile([128, 512], FP16, tag=f"x{d}",
                                    name=f"xt{sb}_{d}")
                nc.sync.dma_start(
                    xt_t,
                    xt_d[d * 128:(d + 1) * 128, sb * 512:(sb + 1) * 512])
                xts[sb][d] = xt_t
            x8_t = xt8_pool.tile([128, DCH, 512], FP8, tag="x8",
                                 name=f"xt8_{sb}")
            nc.sync.dma_start(
                x8_t, xt8_d[:, sb * DCH:(sb + 1) * DCH, :].bitcast(FP8))
            xt8s[sb] = x8_t

        # wf3 [128, NI, F] fp16: row a*64+p, chunk i -> W_fin[(2i+a)*64+p, :].
        # Emitted after xt so the x stream wins the DMA queues.
        wf3 = w_pool.tile([128, NI, F], FP16, tag="wf3")
        for i in range(4):
            nc.sync.dma_start(wf3[:, 2 * i:2 * i + 2, :],
                              wf3_d[:, 2 * i:2 * i + 2, :])

        for sb in range(NSB):
            cols = slice(sb * 512, (sb + 1) * 512)
            for w, dst in ((wk, kt2), (wq, qt2)):
                if KLEVEL < 1:
                    break
                ps_p = pp_pool.tile([128, 512], FP32, tag="pj",
                                    name=f"pp{sb}_{0 if dst is kt2 else 1}")
                for d in range(DCH):
                    nc.tensor.matmul(ps_p, w[:, d * 128:(d + 1) * 128],
                                     xts[sb][d],
                                     start=(d == 0), stop=(d == DCH - 1))
                nc.vector.tensor_copy(dst[:, cols], ps_p)
            # natural V via fp8 DoubleRow: per t-tile, K=256 per pass
            for jj in range(4):
                if KLEVEL < 1:
                    break
                j = sb * 4 + jj
                t0 = jj * 128
                pv = pp_pool.tile([128, 256], FP32, tag="pv", name=f"pv{j}")
                for m in range(DCH // 2):
                    nc.tensor.matmul(pv,
                                     xt8s[sb][:, 2 * m:2 * m + 2, t0:t0 + 128],
                                     wv8[:, 2 * m:2 * m + 2, :],
                                     start=(m == 0), stop=(m == DCH // 2 - 1),
                                     perf_mode=DR)
                nc.vector.tensor_copy(v2n[:, j:j + 1, :], pv)
            # q0/half0 scores over this s-block's freshly-projected t-tiles
            for jp in (2 * sb, 2 * sb + 1):
                emit_scores_attn(0, jp, ps_a_q0, sc_pool)

    pp_pool.release()

    # ---------------- stage S + F: remaining scores / attn / final ----------
    with tc.tile_pool(name="ps_f", bufs=2, space="PSUM", side="right") as ps_f_pool:

        emit_at2q_copies(at2q0, ps_a_q0, 0)
        # q0 half1
        ps_a = alloc_ps_a(1)
        for jp in range(NT // 2):
            emit_scores_attn(1, jp, ps_a, sc_pool)
        emit_at2q_copies(at2q0, ps_a, 1)

        # q1 half0 first, THEN final(q0): the final-linear matmuls overlap
        # the q1 sigmoid stream instead of stalling the ACT seam
        at2q1 = at_pool.tile([128, NI, 128], FP16, tag="at", name="at1")
        ps_a = alloc_ps_a(2)
        for jp in range(NT // 2):
            emit_scores_attn(2, jp, ps_a, sc_pool)
        emit_at2q_copies(at2q1, ps_a, 0)
        emit_final(0, at2q0, ps_f_pool)

        ps_a = alloc_ps_a(3)
        for jp in range(NT // 2):
            emit_scores_attn(3, jp, ps_a, sc_pool)
        emit_at2q_copies(at2q1, ps_a, 1)
        emit_final(1, at2q1, ps_f_pool)


def build_bass(replicas: int = 1) -> bass.Bass:
    nc = bacc.Bacc("TRN2", target_bir_lowering=False, debug=False,
                   num_devices=NCORES)
    xt_d = nc.dram_tensor("xt", [D, S], FP16, kind="ExternalInput").ap()
    xt8_d = nc.dram_tensor("xt8", [128, NSB * DCH, 512], mybir.dt.uint8,
                           kind="ExternalInput").ap()
    wq_d = nc.dram_tensor("wq", [128, D], FP16, kind="ExternalInput").ap()
    wk_d = nc.dram_tensor("wk", [128, D], FP16, kind="ExternalInput").ap()
    wv8_d = nc.dram_tensor("wv8", [128, DCH, 256], mybir.dt.uint8,
                           kind="ExternalInput").ap()
    wf3_d = nc.dram_tensor("wf3", [128, NI, F], FP16,
                           kind="ExternalInput").ap()
    out_d = nc.dram_tensor("out", [HL * 128, F], FP16,
                           kind="ExternalOutput").ap()
    with tile.TileContext(nc) as tc:
        for _ in range(replicas):
            with ExitStack() as ctx:
                build_kernel(ctx, tc, xt_d, xt8_d, wq_d, wk_d, wv8_d, wf3_d,
                             out_d)
    nc.finalize()
    return nc


_NC_CACHE = None
_EXEC_CACHE = None
LAST_DEV_ARGS = None
LAST_OUT_NAMES = None


def _get_nc():
    global _NC_CACHE
    if _NC_CACHE is None:
        _NC_CACHE = build_bass()
    return _NC_CACHE


def _get_executor():
    """Compile the SPMD PJRT executable once (mirrors bass2jax.run_bass_via_pjrt,
    minus output-buffer donation)."""
    global _EXEC_CACHE
    if _EXEC_CACHE is not None:
        return _EXEC_CACHE
    import concourse.mybir as mybir

    nc = _get_nc()
    install_neuronx_cc_hook()
    partition_name = (nc.partition_id_tensor.name
                      if nc.partition_id_tensor else None)
    in_names, out_names, out_avals = [], [], []
    out_shapes = []
    for alloc in nc.m.functions[0].allocations:
        if not isinstance(alloc, mybir.MemoryLocationSet):
            continue
        name = alloc.memorylocations[0].name
        if alloc.kind == "ExternalInput":
            if name != partition_name:
                in_names.append(name)
        elif alloc.kind == "ExternalOutput":
            shape = tuple(alloc.tensor_shape)
            dtype = mybir.dt.np(alloc.dtype)
            out_names.append(name)
            out_avals.append(jax.core.ShapedArray(shape, dtype))
            out_shapes.append((shape, dtype))
    n_params = len(in_names)
    all_names = list(in_names) + list(out_names)
    if partition_name is not None:
        all_names.append(partition_name)

    def _body(*args):
        operands = list(args)
        if partition_name is not None:
            operands.append(partition_id_tensor())
        outs = _bass_exec_p.bind(
            *operands,
            out_avals=tuple(out_avals),
            in_names=tuple(all_names),
            out_names=tuple(out_names),
            lowering_input_output_aliases=(),
            sim_require_finite=True,
            sim_require_nnan=True,
            nc=nc,
        )
        return tuple(outs)

    devices = jax.devices()[:NCORES]
    mesh = Mesh(np.asarray(devices), ("core",))
    n_args = n_params + len(out_names)
    sharded = jax.jit(shard_map(
        _body, mesh=mesh,
        in_specs=(PartitionSpec("core"),) * n_args,
        out_specs=(PartitionSpec("core"),) * len(out_names),
        check_rep=False))
    _EXEC_CACHE = (sharded, mesh, in_names, out_names, out_shapes)
    return _EXEC_CACHE


def _run_spmd(in_maps):
    """Execute on all cores; returns list of per-core {name: np.ndarray}."""
    global LAST_DEV_ARGS, LAST_OUT_NAMES
    sharded, mesh, in_names, out_names, out_shapes = _get_executor()
    sh = NamedSharding(mesh, PartitionSpec("core"))
    args = [np.concatenate([im[name] for im in in_maps], axis=0)
            for name in in_names]
    for shape, dtype in out_shapes:
        args.append(np.zeros((NCORES * shape[0],) + shape[1:], dtype))
    dev_args = [jax.device_put(a, sh) for a in args]
    LAST_DEV_ARGS = dev_args
    LAST_OUT_NAMES = out_names
    outs = sharded(*dev_args)
    jax.block_until_ready(outs)
    results = []
    for c in range(NCORES):
        res = {}
        for i, name in enumerate(out_names):
            g = np.asarray(outs[i])
            d0 = g.shape[0] // NCORES
            res[name] = g[c * d0:(c + 1) * d0]
        results.append(res)
    return results


def bench(iters=32):
    import time
    sharded = _get_executor()[0]
    assert LAST_DEV_ARGS is not None, "call kernel() first"
    outs = sharded(*LAST_DEV_ARGS)
    jax.block_until_ready(outs)
    t0 = time.perf_counter()
    pend = [sharded(*LAST_DEV_ARGS) for _ in range(iters)]
    jax.block_until_ready(pend)
    return (time.perf_counter() - t0) / iters


_FAST_CACHE = None


def _get_fast():
    global _FAST_CACHE
    if _FAST_CACHE is not None:
        return _FAST_CACHE
    _FAST_CACHE = _make_fast_for(_get_nc())
    return _FAST_CACHE


def bench_fast(iters=64):
    import time
    fn = _get_fast()
    assert LAST_DEV_ARGS is not None
    outs = fn(*LAST_DEV_ARGS)
    jax.block_until_ready(outs)
    best = None
    for _ in range(3):
        t0 = time.perf_counter()
        pend = [fn(*LAST_DEV_ARGS) for _ in range(iters)]
        jax.block_until_ready(pend)
        dt = (time.perf_counter() - t0) / iters
        best = dt if best is None else min(best, dt)
    return best


def _make_fast_for(nc):
    from concourse.bass2jax import fast_dispatch_compile
    import concourse.mybir as mybir

    install_neuronx_cc_hook()
    pn = nc.partition_id_tensor.name if nc.partition_id_tensor else None
    in_names, out_names, out_avals = [], [], []
    for alloc in nc.m.functions[0].allocations:
        if not isinstance(alloc, mybir.MemoryLocationSet):
            continue
        name = alloc.memorylocations[0].name
        if alloc.kind == "ExternalInput":
            if name != pn:
                in_names.append(name)
        elif alloc.kind == "ExternalOutput":
            out_names.append(name)
            out_avals.append(jax.core.ShapedArray(
                tuple(alloc.tensor_shape), mybir.dt.np(alloc.dtype)))
    all_names = list(in_names) + list(out_names)
    if pn:
        all_names.append(pn)

    def _body(*a):
        ops = list(a)
        if pn:
            ops.append(partition_id_tensor())
        return tuple(_bass_exec_p.bind(
            *ops, out_avals=tuple(out_avals), in_names=tuple(all_names),
            out_names=tuple(out_names), lowering_input_output_aliases=(),
            sim_require_finite=True, sim_require_nnan=True, nc=nc))

    mesh = Mesh(np.asarray(jax.devices()[:NCORES]), ("core",))
    na = len(in_names) + len(out_names)

    def cf():
        return jax.jit(shard_map(
            _body, mesh=mesh,
            in_specs=(PartitionSpec("core"),) * na,
            out_specs=(PartitionSpec("core"),) * len(out_names),
            check_rep=False)).lower(*LAST_DEV_ARGS).compile()

    return fast_dispatch_compile(cf)


def bench_body(rounds=9, n=96, r_lo=3, r_hi=15):
    """Kernel-body execution time via the replica-count slope: per-exec wall
    time of an r_hi-body NEFF minus an r_lo-body NEFF, divided by the body
    delta. Rounds alternate lo/hi so slow tunnel drift cancels; the median
    difference is returned. Multi-body NEFFs keep the device busy well past
    the ~1 ms axon dispatch overhead."""
    import time
    import statistics
    assert LAST_DEV_ARGS is not None, "call kernel() first"
    fn_lo = _make_fast_for(build_bass(replicas=r_lo))
    fn_hi = _make_fast_for(build_bass(replicas=r_hi))
    jax.block_until_ready(fn_lo(*LAST_DEV_ARGS))
    jax.block_until_ready(fn_hi(*LAST_DEV_ARGS))

    def run(fn, m):
        t0 = time.perf_counter()
        pend = [fn(*LAST_DEV_ARGS) for _ in range(m)]
        jax.block_until_ready(pend)
        return (time.perf_counter() - t0) / m

    run(fn_lo, 16)
    run(fn_hi, 16)
    diffs = []
    for _ in range(rounds):
        lo = run(fn_lo, n)
        hi = run(fn_hi, n)
        diffs.append((hi - lo) / (r_hi - r_lo))
    return statistics.median(diffs)


def _np_fp8():
    return mybir.dt.np(FP8)


def _layout_w(w, c):
    """[H, D, P] global weights -> per-core [128, D] stationary layout:
    out[di, dc*128 + (h*64+p)] = w[2c+h, dc*128+di, p]"""
    wl = np.transpose(w[HL * c:HL * (c + 1)], (1, 0, 2)).reshape(D, P2)
    wl = wl.reshape(DCH, 128, P2).transpose(1, 0, 2).reshape(128, DCH * P2)
    return np.ascontiguousarray(wl, dtype=np.float16)


def make_in_maps(x, Qw, Kw, Vw, W_fin):
    f8 = _np_fp8()
    xt16 = np.ascontiguousarray(x.T.astype(np.float16))          # [D, S]
    # xt8[di, sb*8+ch, s'] = x[sb*512+s', ch*128+di]
    x8 = x.astype(f8)                                            # [S, D]
    xt8 = np.ascontiguousarray(
        x8.reshape(NSB, 512, DCH, 128).transpose(3, 0, 2, 1)
        .reshape(128, NSB * DCH, 512)).view(np.uint8)
    # wf3[a*64+p, i, f] = W_fin[(2i+a)*64+p, f]
    wf = W_fin.reshape(NI, 2, 64, F).transpose(1, 2, 0, 3)       # [a, p, i, f]
    wf3 = np.ascontiguousarray(wf.reshape(128, NI, F).astype(np.float16))

    in_maps = []
    for c in range(NCORES):
        # wv8[di, ch, q2*128 + dup*64 + p] = Vw[2c+q2, ch*128+di, p]
        vws = Vw[HL * c:HL * c + HL].astype(f8)                  # [2, D, P]
        vv = vws.reshape(HL, DCH, 128, P).transpose(2, 1, 0, 3)  # [di, ch, q2, p]
        wv8 = np.empty((128, DCH, HL, 2, P), f8)
        wv8[:, :, :, 0, :] = vv
        wv8[:, :, :, 1, :] = vv
        wv8 = np.ascontiguousarray(wv8.reshape(128, DCH, 256)).view(np.uint8)
        in_maps.append({
            "xt": xt16,
            "xt8": xt8,
            "wq": _layout_w(Qw, c),
            "wk": _layout_w(Kw, c),
            "wv8": wv8,
            "wf3": wf3,
        })
    return in_maps


def kernel(x, Qw, Kw, Vw, W_fin, b_fin):
    x = np.asarray(x, dtype=np.float32)
    Qw = np.asarray(Qw, dtype=np.float32)
    Kw = np.asarray(Kw, dtype=np.float32)
    Vw = np.asarray(Vw, dtype=np.float32)
    W_fin = np.asarray(W_fin, dtype=np.float32)
    b_fin = np.asarray(b_fin, dtype=np.float32)

    in_maps = make_in_maps(x, Qw, Kw, Vw, W_fin)
    results = _run_spmd(in_maps)
    out = np.concatenate([results[c]["out"] for c in range(NCORES)], axis=0)
    return (out + b_fin).astype(np.float32)
